# revision 27
# baseline (speedup 1.0000x reference)
"""ConvLSTM (nn_BottomConvLSTM) Trainium2 Bass kernel — Winograd F(2,3) over W.

Problem (hardcoded):
  x:       [T=12, B=2, C=64, H=128, W=128] f32
  W_gates: [512, 192, 3, 3] f32,  b_gates: [512] f32
  W_out:   [64, 128, 3, 3] f32,   b_out:   [64] f32
  out:     [T, B, 64, H, W] f32

Sharding: 8 cores = B(2) x H-slabs(4 x 32 rows); the T recurrence stays
on-chip per core.  Each step exchanges one boundary row per direction with
slab neighbors via a per-step DRAM AllGather over the 4-core B-group,
reduced into the halo rows with per-core one-hot masks (zero at the global
image boundary = SAME zero padding).

All three convs (gates-x, gates-h, out) run as 1D Winograd F(2,3) along W:
each 3-tap row conv becomes 4 "slot" matmuls on transformed inputs at half
the column count, cutting PE cycles ~1.4x (the direct-conv kernel is >96%
PE-busy at the power-throttled ~1.95 GHz clock, so cycles == time).  Slot
products M0..M3 accumulate in PSUM; 8-row tiles make each M plane exactly
one bank [128,512].  The inverse transform (even = M0+M1+M2, odd =
M1-M2-M3) happens at evacuation: ScalarE seeds even/odd with
Identity(M0/M1 + bias), VectorE adds/subtracts M2/M3 (mixed PSUM+SBUF
operands only — PSUM has a single DVE read port).

h and c live column-de-interleaved (even|odd planes) so the winograd input
transform of h (4 unit-stride DVE ops per row block) and the LSTM pointwise
(on GPSIMD, keeping DVE free for evacuation) never need strided columns.
x and all weights are transformed on the host.  The h transform is computed
once per step and shared by the gates (3 dy taps) and the fused out conv.
"""

import os
import sys

import numpy as np

T = 12
CIN = 64
HID = 128
H_FULL = 128
W = 128
NB = 2
NSLAB = 4
SLAB = H_FULL // NSLAB  # 32
WP = W + 2  # de-interleaved padded width: [HE(65) | HO(65)]
Q = W // 2  # 64 column pairs

HROWS = SLAB + 2  # h buffer rows: halo -1 | own 0..31 | halo 32  (= 34)
XWROWS = SLAB + 1  # transformed-x buffer rows 0..32 (low = spatial -1..31)

N_CORES = 8
LAST_EXEC_NS = None

FAST_DTYPE = os.environ.get("KERNEL_MM_DTYPE", "bfloat16")


def _mm_np(a):
    """Cast a host array to the matmul input dtype."""
    if FAST_DTYPE == "bfloat16":
        import ml_dtypes

        return np.ascontiguousarray(a.astype(ml_dtypes.bfloat16))
    return np.ascontiguousarray(a, dtype=np.float32)


def _import_concourse():
    try:
        import concourse.bass  # noqa: F401
        return
    except ImportError:
        pass
    for p in ("/opt/trn_rl_repo", "/root/.axon_site/_ro/trn_rl_repo"):
        if os.path.isdir(p) and p not in sys.path:
            sys.path.insert(0, p)
    import concourse.bass  # noqa: F401


def build_nc(t_steps=T, slab=SLAB):
    _import_concourse()
    import concourse.tile as tile
    from concourse import bacc, mybir

    F32 = mybir.dt.float32
    FMM = getattr(mybir.dt, FAST_DTYPE)
    AF = mybir.ActivationFunctionType
    SUB = mybir.AluOpType.subtract

    nc = bacc.Bacc("TRN2", target_bir_lowering=False, debug=False)
    # host-transformed x: [T, 128(dy-packed ch), XWROWS, 4 slots, Q]
    xp = nc.dram_tensor("xp", [t_steps, 128, XWROWS, 4, Q], FMM, kind="ExternalInput").ap()
    whd = nc.dram_tensor("wh", [128, 3, 4, 512], FMM, kind="ExternalInput").ap()
    wxd = nc.dram_tensor("wx", [128, 4, 512], FMM, kind="ExternalInput").ap()
    wx2d = nc.dram_tensor("wx2", [128, 4, 512], FMM, kind="ExternalInput").ap()
    wod = nc.dram_tensor("wo", [128, 3, 4, 64], FMM, kind="ExternalInput").ap()
    bgd = nc.dram_tensor("bg", [128, 4], F32, kind="ExternalInput").ap()
    bod = nc.dram_tensor("bo", [128, 1], F32, kind="ExternalInput").ap()
    # one-hot gather-block masks: hm[:, j, 0, :] selects block j for halo
    # row -1; hm[:, j, 1, :] selects block j for halo row 32.
    hmd = nc.dram_tensor("hm", [128, 4, 2, WP], FMM, kind="ExternalInput").ap()
    # output stays column-de-interleaved ([even|odd] plane per row); the
    # host re-interleaves for free.  Strided (0::2) DRAM writes would
    # degenerate into one DMA packet per element.
    out = nc.dram_tensor("out", [t_steps, 64, slab, 2, Q], F32, kind="ExternalOutput").ap()
    # per-step exchange bounces (collectives need Internal DRAM)
    bin_ = nc.dram_tensor("hbin", [t_steps, 128, 2, WP], FMM).ap()
    bout = nc.dram_tensor("hbout", [t_steps, 4, 128, 2, WP], FMM).ap()
    # tiny warmup-collective buffers: establish the CC channels early
    dwin = nc.dram_tensor("dwin", [128, 16], FMM).ap()
    dwout = nc.dram_tensor("dwout", [4, 128, 16], FMM).ap()

    groups = [[0, 1, 2, 3], [4, 5, 6, 7]]

    with tile.TileContext(nc) as tc:
        with (
            tc.tile_pool(name="pw", bufs=1) as pw,
            tc.tile_pool(name="pstate", bufs=1) as pstate,
            tc.tile_pool(name="px", bufs=3) as px,
            tc.tile_pool(name="ppre", bufs=8) as ppre,
            tc.tile_pool(name="ptmp", bufs=10) as ptmp,
            tc.tile_pool(name="prx", bufs=2) as prx,
            tc.tile_pool(name="pout", bufs=2) as pout,
            tc.tile_pool(name="pps", bufs=2, space="PSUM") as pps,
        ):
            # Warm the PE clock with dummy matmuls while weight DMAs fly.
            warm = pw.tile([128, 640], FMM, tag="warm", name="warm")
            nc.vector.memset(warm[:], 0)
            wps = pps.tile([128, 4, 512], F32, tag="ps", name="warm_ps")
            for k in range(16):
                nc.tensor.matmul(
                    wps[:, 0, :], warm[:, 0:128], warm[:, 128:640],
                    start=(k == 0), stop=(k == 15),
                )
            nc.gpsimd.collective_compute(
                "AllGather",
                mybir.AluOpType.bypass,
                replica_groups=groups,
                ins=[dwin],
                outs=[dwout],
            )

            wh_sb = pw.tile([128, 3, 4, 512], FMM, tag="wh", name="wh_sb")
            # per-slot x-weight tiles: step 1's first matmuls start as soon
            # as their own 128 KB chunk lands, not the whole weight bulk
            wx_sb = [
                pw.tile([128, 512], FMM, tag=f"wx{k}", name=f"wx_sb{k}")
                for k in range(4)
            ]
            wx2_sb = [
                pw.tile([128, 512], FMM, tag=f"wx2{k}", name=f"wx2_sb{k}")
                for k in range(4)
            ]
            wo_sb = pw.tile([128, 3, 4, 64], FMM, tag="wo", name="wo_sb")
            bg_sb = pw.tile([128, 4], F32, tag="bg", name="bg_sb")
            bo_sb = pw.tile([128, 1], F32, tag="bo", name="bo_sb")
            hm_sb = pw.tile([128, 4, 2, WP], FMM, tag="hm", name="hm_sb")

            # x-weights first: step 1's matmuls need only wx/wx2 + the
            # first x tile, so they can start before the weight bulk lands.
            # Startup DMAs spread across engine queues to run in parallel.
            nc.sync.dma_start(wx_sb[0][:], wxd[:, 0, :])
            nc.gpsimd.dma_start(wx2_sb[0][:], wx2d[:, 0, :])
            pre_xw = px.tile([128, 9, 4, Q], FMM, tag="xs", name="xs")
            nc.scalar.dma_start(pre_xw[:], xp[0, :, 0:9, :, :])
            for k in range(1, 4):
                nc.sync.dma_start(wx_sb[k][:], wxd[:, k, :])
                nc.gpsimd.dma_start(wx2_sb[k][:], wx2d[:, k, :])
            nc.sync.dma_start(bg_sb[:], bgd[:])
            nc.sync.dma_start(bo_sb[:], bod[:])
            nc.gpsimd.dma_start(wo_sb[:], wod[:])
            nc.gpsimd.dma_start(hm_sb[:], hmd[:])
            nc.sync.dma_start(wh_sb[:], whd[:])

            # state: h de-interleaved [HE(65)|HO(65)] per row; pad entries
            # (HE[64]=col 128, HO[0]=col -1) are never written, stay zero.
            h_a = pstate.tile([128, HROWS, WP], FMM, tag="ha", name="h_a")
            h_b = pstate.tile([128, HROWS, WP], FMM, tag="hb", name="h_b")
            # winograd-transformed h, 4 slots per row
            hw_a = pstate.tile([128, HROWS, 4, Q], FMM, tag="hwa", name="hw_a")
            hw_b = pstate.tile([128, HROWS, 4, Q], FMM, tag="hwb", name="hw_b")
            c_sb = pstate.tile([128, 2, slab, Q], F32, tag="c", name="c_sb")
            nc.vector.memset(h_a[:], 0)
            nc.vector.memset(h_b[:], 0)
            nc.vector.memset(hw_a[:], 0)
            nc.vector.memset(hw_b[:], 0)
            h_tiles = [h_a, h_b]
            hw_tiles = [hw_a, hw_b]

            def emit_htw(h_t, hw_t, r0, nr):
                """Winograd input transform for h buffer rows r0..r0+nr-1.

                D0 = HO[q]-HO[q+1]; D1 = HE[q]+HO[q+1];
                D2 = HO[q+1]-HE[q]; D3 = HE[q]-HE[q+1]
                (HE = cols 0:65, HO[j] = col 2j-1 at cols 65:130).
                """
                rs = slice(r0, r0 + nr)
                he = h_t[:, rs, 0:Q]
                he1 = h_t[:, rs, 1 : Q + 1]
                ho = h_t[:, rs, 65 : 65 + Q]
                ho1 = h_t[:, rs, 66 : 66 + Q]
                # on GPSIMD: keeps the DVE FIFO free for PSUM evacuations
                nc.gpsimd.tensor_tensor(hw_t[:, rs, 0, :], ho, ho1, SUB)
                nc.gpsimd.tensor_add(hw_t[:, rs, 1, :], he, ho1)
                nc.gpsimd.tensor_tensor(hw_t[:, rs, 2, :], ho1, he, SUB)
                nc.gpsimd.tensor_tensor(hw_t[:, rs, 3, :], he, he1, SUB)

            def evac(pt, dest_eo, bias):
                """Inverse transform even=M0+M1+M2, odd=M1-M2-M3 (+bias).

                dest_eo: [128, 2, 8, Q] SBUF tile.  One merged ACT Identity
                seeds even|odd from the contiguous M0|M1 banks; DVE then
                adds/subtracts M2/M3 (mixed PSUM+SBUF operands only).
                """
                dest_e = dest_eo[:, 0, :, :]
                dest_o = dest_eo[:, 1, :, :]
                nc.scalar.activation(dest_eo[:], pt[:, 0:2, :], AF.Identity, bias=bias)
                nc.vector.tensor_add(dest_e, dest_e, pt[:, 1, :])
                nc.vector.tensor_add(dest_e, dest_e, pt[:, 2, :])
                nc.vector.tensor_tensor(dest_o, dest_o, pt[:, 2, :], SUB)
                nc.vector.tensor_tensor(dest_o, dest_o, pt[:, 3, :], SUB)

            def emit_out_pair(tt, hw_t, yo):
                """Winograd out-conv pair: rows yo..yo+7 (A) | yo+8..yo+15 (B)
                run concurrently in the two PE column halves."""
                po = pps.tile([128, 4, 512], F32, tag="ps", name="po")
                for k in range(4):
                    for dy in range(3):
                        nc.tensor.matmul(
                            po[0:64, k, :],
                            wo_sb[:, dy, k, :],
                            hw_t[:, yo + dy : yo + dy + 8, k, :],
                            start=(dy == 0), stop=(dy == 2),
                            tile_position=(0, 0),
                        )
                        nc.tensor.matmul(
                            po[64:128, k, :],
                            wo_sb[:, dy, k, :],
                            hw_t[:, yo + 8 + dy : yo + 8 + dy + 8, k, :],
                            start=(dy == 0), stop=(dy == 2),
                            tile_position=(0, 64),
                        )
                ob = pout.tile([128, 2, 8, Q], F32, tag="ostage", name="ob")
                evac(po, ob, bo_sb[:, 0:1])
                # contiguous de-interleaved DRAM writes (planes last)
                for p in range(2):
                    nc.sync.dma_start(
                        out[tt - 1, :, yo : yo + 8, p, :], ob[0:64, p, :, :]
                    )
                    nc.sync.dma_start(
                        out[tt - 1, :, yo + 8 : yo + 16, p, :], ob[64:128, p, :, :]
                    )

            def make_tail(t, y, pres, h_cur, hw_cur):
                """Pointwise tail of a gate tile, emitted one tile later so
                the ACT-FIFO (tct waits the DVE c-chain) never blocks the
                next tile's PSUM-evacuation seeds."""

                def tail():
                    cw = c_sb[:, :, y : y + 8, :]
                    tg = ptmp.tile([128, 2, 8, Q], F32, tag="tmp", name="tg")
                    nc.scalar.activation(tg[:], pres[3][:], AF.Tanh)
                    si = ptmp.tile([128, 2, 8, Q], F32, tag="tmp", name="si")
                    nc.scalar.activation(si[:], pres[0][:], AF.Sigmoid)
                    so = ptmp.tile([128, 2, 8, Q], F32, tag="tmp", name="so")
                    if t == 1:
                        nc.scalar.activation(so[:], pres[2][:], AF.Sigmoid)
                        nc.gpsimd.tensor_mul(cw, si[:], tg[:])
                    else:
                        sf = ptmp.tile([128, 2, 8, Q], F32, tag="tmp", name="sf")
                        nc.scalar.activation(sf[:], pres[1][:], AF.Sigmoid)
                        nc.scalar.activation(so[:], pres[2][:], AF.Sigmoid)
                        pr = ptmp.tile([128, 2, 8, Q], F32, tag="tmp", name="pr")
                        nc.gpsimd.tensor_mul(pr[:], si[:], tg[:])
                        nc.gpsimd.tensor_mul(cw, cw, sf[:])
                        nc.gpsimd.tensor_add(cw, cw, pr[:])
                    tct = ptmp.tile([128, 2, 8, Q], F32, tag="tmp", name="tct")
                    nc.scalar.activation(tct[:], cw, AF.Tanh)
                    # h = so*tct into the de-interleaved planes (spatial rows
                    # y..y+7 = buffer rows y+1..y+8)
                    nc.gpsimd.tensor_mul(
                        h_cur[:, y + 1 : y + 9, 0:Q], so[:, 0, :, :], tct[:, 0, :, :]
                    )
                    nc.gpsimd.tensor_mul(
                        h_cur[:, y + 1 : y + 9, 66 : 66 + Q], so[:, 1, :, :], tct[:, 1, :, :]
                    )
                    # transform the rows just written (shared gates + out)
                    emit_htw(h_cur, hw_cur, y + 1, 8)
                    if y == 0:
                        # spatial row 0 -> above neighbor's halo row 32
                        nc.gpsimd.dma_start(bin_[t - 1, :, 1:2, :], h_cur[:, 1:2, :])
                    elif y == slab - 8:
                        # spatial row 31 -> below neighbor's halo row -1
                        nc.gpsimd.dma_start(
                            bin_[t - 1, :, 0:1, :], h_cur[:, slab : slab + 1, :]
                        )

                return tail

            # 8-row gate tiles, edge tiles first: their (deferred) tails
            # carry the halo-row sends, so the AllGather triggers mid-step
            # and its latency hides under the remaining tiles.
            tile_order = [0, 24, 8, 16]

            for t in range(1, t_steps + 1):
                h_cur = h_tiles[(t - 1) % 2]
                hw_cur = hw_tiles[(t - 1) % 2]
                hw_prev = hw_tiles[t % 2]

                pending_tail = None
                for ti, y in enumerate(tile_order):
                    if t == 1 and ti == 0:
                        xw = pre_xw
                    else:
                        xw = px.tile([128, 9, 4, Q], FMM, tag="xs", name="xs")
                        nc.sync.dma_start(xw[:], xp[t - 1, :, y : y + 9, :, :])

                    # gate order g,i,f,o: g's pointwise chain is longest.
                    # x matmuls first so an edge tile's PE work starts
                    # before the previous step's halo transform lands.
                    pres = {}
                    for coc in (3, 0, 1, 2):
                        pt = pps.tile([128, 4, 512], F32, tag="ps", name="ps")
                        cs = slice(coc * 128, (coc + 1) * 128)
                        for k in range(4):
                            nc.tensor.matmul(
                                pt[:, k, :], wx_sb[k][:, cs],
                                xw[:, 0:8, k, :],
                                start=True, stop=False,
                            )
                            nc.tensor.matmul(
                                pt[:, k, :], wx2_sb[k][:, cs],
                                xw[:, 1:9, k, :],
                                start=False, stop=(t == 1),
                            )
                        if t > 1:
                            for k in range(4):
                                for dy in range(3):
                                    nc.tensor.matmul(
                                        pt[:, k, :], wh_sb[:, dy, k, cs],
                                        hw_prev[:, y + dy : y + dy + 8, k, :],
                                        start=False, stop=(dy == 2),
                                    )
                        pe = ppre.tile([128, 2, 8, Q], F32, tag="pre", name="pre")
                        pres[coc] = pe
                        evac(pt, pe, bg_sb[:, coc : coc + 1])

                    if ti == 1 and t > 1:
                        # deferred out-conv pairs of step t-1 (need halos).
                        # Emitted BEFORE the tail flush so their seeds are
                        # not stuck behind the tail's activations in the
                        # strict-FIFO ACT queue (their evacuation gates the
                        # next tile's PSUM banks).
                        emit_out_pair(t - 1, hw_prev, 0)
                        emit_out_pair(t - 1, hw_prev, 16)

                    if pending_tail is not None:
                        pending_tail()
                    if t == 1:
                        # step 1 is matmul-light: inline tails move the edge
                        # sends (and so the first exchange) earlier
                        make_tail(t, y, pres, h_cur, hw_cur)()
                    else:
                        pending_tail = make_tail(t, y, pres, h_cur, hw_cur)
                    if ti == 2:
                        # both sends are in flight after tile 24's tail
                        # (emitted in this section); trigger the exchange
                        # now so its latency hides under the rest of the
                        # step instead of queueing behind later gp work.
                        nc.gpsimd.collective_compute(
                            "AllGather",
                            mybir.AluOpType.bypass,
                            replica_groups=groups,
                            ins=[bin_[t - 1]],
                            outs=[bout[t - 1]],
                        )

                # reduce the exchanged halo rows, then flush the last tail
                rx = prx.tile([128, 4, 2, WP], FMM, tag="rx", name="rx")
                for j in range(4):
                    nc.sync.dma_start(rx[:, j, :, :], bout[t - 1, j, :, :, :])
                mrx = prx.tile([128, 4, 2, WP], FMM, tag="mrx", name="mrx")
                nc.gpsimd.tensor_mul(mrx[:], rx[:], hm_sb[:])
                t01 = prx.tile([128, 2, WP], FMM, tag="t01", name="t01")
                nc.gpsimd.tensor_add(t01[:], mrx[:, 0, :, :], mrx[:, 1, :, :])
                t23 = prx.tile([128, 2, WP], FMM, tag="t23", name="t23")
                nc.gpsimd.tensor_add(t23[:], mrx[:, 2, :, :], mrx[:, 3, :, :])
                nc.gpsimd.tensor_add(
                    h_cur[:, 0:1, :], t01[:, 0:1, :], t23[:, 0:1, :]
                )
                nc.gpsimd.tensor_add(
                    h_cur[:, slab + 1 : slab + 2, :], t01[:, 1:2, :], t23[:, 1:2, :]
                )
                # transform the halo rows (latency-critical: the next step's
                # first tile reads them), then flush the last tile's tail
                emit_htw(h_cur, hw_cur, 0, 1)
                emit_htw(h_cur, hw_cur, slab + 1, 1)
                if pending_tail is not None:
                    pending_tail()

            emit_out_pair(t_steps, hw_tiles[(t_steps - 1) % 2], 0)
            emit_out_pair(t_steps, hw_tiles[(t_steps - 1) % 2], 16)

    nc.compile()
    return nc


def _gslots(w3):
    """w3: [O, C, 3] taps -> 4 winograd weight slots [4, O, C]."""
    g0, g1, g2 = w3[..., 0], w3[..., 1], w3[..., 2]
    return np.stack([g0, (g0 + g1 + g2) * 0.5, (g0 - g1 + g2) * 0.5, g2])


def prep_weights(W_gates, b_gates, W_out, b_out):
    wg = np.ascontiguousarray(W_gates, dtype=np.float32)  # [512, 192, 3, 3]
    # h weights -> [128cin, 3dy, 4k, 512out]
    whs = np.stack([_gslots(wg[:, CIN:, dy, :]) for dy in range(3)])  # [3,4,512,128]
    wh = np.ascontiguousarray(whs.transpose(3, 0, 1, 2))
    # x dy01 packed: low 64 partitions = dy0 slots, high = dy1
    wx0 = _gslots(wg[:, :CIN, 0, :])  # [4, 512, 64]
    wx1 = _gslots(wg[:, :CIN, 1, :])
    wx = np.ascontiguousarray(
        np.concatenate([wx0.transpose(2, 0, 1), wx1.transpose(2, 0, 1)], axis=0)
    )  # [128, 4, 512]
    wx2s = _gslots(wg[:, :CIN, 2, :])
    wx2 = np.zeros((128, 4, 512), np.float32)
    wx2[64:] = wx2s.transpose(2, 0, 1)
    # out conv -> [128hid, 3dy, 4k, 64out]
    wo3 = np.asarray(W_out, np.float32)  # [64, 128, 3, 3]
    wos = np.stack([_gslots(wo3[:, :, dy, :]) for dy in range(3)])  # [3,4,64,128]
    wo = np.ascontiguousarray(wos.transpose(3, 0, 1, 2))
    bg = np.ascontiguousarray(np.asarray(b_gates, np.float32).reshape(4, 128).T)
    bo = np.ascontiguousarray(
        np.tile(np.asarray(b_out, np.float32).reshape(64, 1), (2, 1))
    )
    return {
        "wh": _mm_np(wh), "wx": _mm_np(wx), "wx2": _mm_np(wx2),
        "wo": _mm_np(wo), "bg": bg, "bo": bo,
    }


def prep_hm(s):
    """One-hot gather-block masks for slab index s (group-local rank).

    hm[:, j, 0, :] = 1 iff block j is my upper neighbor (s-1): halo row -1.
    hm[:, j, 1, :] = 1 iff block j is my lower neighbor (s+1): halo row 32.
    Zero at the global image boundary (SAME zero padding).
    """
    hm = np.zeros((128, 4, 2, WP), np.float32)
    if s > 0:
        hm[:, s - 1, 0, :] = 1.0
    if s < NSLAB - 1:
        hm[:, s + 1, 1, :] = 1.0
    return {"hm": _mm_np(hm)}


def prep_x(x, t_steps=T, slab=SLAB, h_img=H_FULL):
    """x -> per-core winograd-transformed packed [T, 128, XWROWS, 4, Q].

    Core c = b * NSLAB + s covers global rows [slab*s, slab*s + slab).
    Buffer row r: partitions 0:64 = D-slots of x row slab*s + r - 1,
    partitions 64:128 = the same one row down.
    """
    x = np.asarray(x, np.float32)
    tt, nb = x.shape[0], x.shape[1]
    # rows: xpad row i = spatial row i-2 (so rows -2..h_img+1 exist);
    # cols: xpad col j = spatial col j-1
    xpad = np.zeros((tt, nb, CIN, h_img + 4, W + 2), np.float32)
    xpad[:, :, :, 2 : 2 + h_img, 1 : 1 + W] = x
    E = xpad[:, :, :, :, 1 : W + 2 : 2]  # cols 0,2,..,W   (Q+1 entries)
    O = xpad[:, :, :, :, 0 : W + 1 : 2]  # cols -1,1,..,W-1 (Q+1 entries)
    D = np.stack(
        [
            O[..., :-1] - O[..., 1:],
            E[..., :-1] + O[..., 1:],
            O[..., 1:] - E[..., :-1],
            E[..., :-1] - E[..., 1:],
        ],
        axis=4,
    )  # [T, B, C, rows, 4, Q]
    cores = []
    for b in range(nb):
        for s in range(NSLAB):
            r0 = slab * s  # buffer row 0 low = spatial r0-1 = xpad row r0+1
            lower = D[:, b, :, r0 + 1 : r0 + 1 + XWROWS]
            upper = D[:, b, :, r0 + 2 : r0 + 2 + XWROWS]
            cores.append(_mm_np(np.concatenate([lower, upper], axis=1)))
    return cores


_NC_CACHE = {}


def _get_nc():
    key = (T, SLAB, FAST_DTYPE)
    if key not in _NC_CACHE:
        _NC_CACHE[key] = build_nc(T, SLAB)
    return _NC_CACHE[key]


def kernel(x, W_gates, b_gates, W_out, b_out):
    _import_concourse()
    from concourse.bass_utils import run_bass_kernel_spmd

    nc = _get_nc()
    wmap = prep_weights(W_gates, b_gates, W_out, b_out)
    xcores = prep_x(x)
    in_maps = []
    for c, xc in enumerate(xcores):
        s = c % NSLAB
        in_maps.append(dict(wmap, xp=xc, **prep_hm(s)))

    trace = bool(os.environ.get("KERNEL_TRACE"))
    kwargs = {}
    if trace:
        kwargs = {"trace": True, "tmpdir": os.environ.get("KERNEL_TRACE_DIR") or None}
    res = run_bass_kernel_spmd(nc, in_maps, core_ids=list(range(N_CORES)), **kwargs)
    if trace:
        global LAST_EXEC_NS
        LAST_EXEC_NS = res.exec_time_ns
        print(f"HW exec time: {res.exec_time_ns} ns")

    out = np.empty((T, NB, CIN, H_FULL, W), np.float32)
    for c in range(N_CORES):
        b, s = divmod(c, NSLAB)
        o = res.results[c]["out"]  # [T, 64, slab, 2, Q] de-interleaved
        out[:, b, :, SLAB * s : SLAB * (s + 1), 0::2] = o[:, :, :, 0, :]
        out[:, b, :, SLAB * s : SLAB * (s + 1), 1::2] = o[:, :, :, 1, :]
    return out
